# revision 52
# baseline (speedup 1.0000x reference)
"""MultiHeadAttention Trainium2 kernel (8-core SPMD, head/batch sharded).

Reference semantics (E=1024, H=16, D=64, B=2, S=2048):
    qp = (q @ wq.T + bq).reshape(B, H, S, D)   # RAW view, not transpose!
    scores = qp @ kp^T * 1/sqrt(E); attn = softmax(scores)
    out = (attn @ vp).reshape(B, S, E) @ wo.T + bo

Because the reshape is a raw view, head h of batch b corresponds to the
contiguous 128-row block rows[128h:128h+128] of the projected [S, E]
matrix, viewed as [2048, 64].  Each core therefore only needs 512 rows of
q/k/v (4 heads) plus the full weight matrices.

Inside each head we use the permuted sequence order i' = 128r + a
(original in-head index i = 16a + r, a=row-in-block 0..127, r=col-block
0..15).  This is a symmetric permutation of Q/K/V rows, so softmax+AV
commute with it; it makes every layout matmul-native.

v2 pipeline notes (vs the v1 baseline at 354us):
  * v1 kept the PE waiting on exp every score chunk (PSUM ring depth 1),
    so the HAM clock gate never saw 3.4us of unbroken PE work and the
    whole attention phase ran at K=4/8 (1.2 GHz).  v2 processes 512
    queries per unit so a score chunk is [128, 2, 512] = 2 PSUM banks,
    the ring holds 3 chunks, and QK runs RUNAHEAD chunks ahead of AV.
  * exp alternates whole chunks between ScalarE (true exp) and VectorE
    (bit-trick exp2); softmax tails use one merged [65, 512] drain copy
    plus reciprocal_approx_fast, with the normalize mults on GpSimd.
  * all weights/x prefetch up-front in per-k-chunk tiles: weights on the
    sync queue, x + small tensors on gpsimd, staging shuffles on the
    vector queue, y stores on scalar — no queue blocks another phase.
  * biases ride the PSUM drain copies (per-partition tensor_scalar for
    the transposed Q/K layout, partition-broadcast tensor_tensor for
    V/out), not 512-cycle K=1 matmuls.
"""

import numpy as np

import concourse.bass as bass
import concourse.mybir as mybir
import concourse.tile as tile
from concourse import bacc
from concourse.bass_utils import run_bass_kernel_spmd

B, S, E = 2, 2048, 1024
H, D = 16, 64
HEADS_PER_CORE = 4
ROWS = 512  # rows of the [S,E] projected matrix handled per core
N_CORES = 8
SCALE = 1.0 / float(np.sqrt(np.float32(E)))

F32 = mybir.dt.float32
BF16 = mybir.dt.bfloat16
I16 = mybir.dt.int16
F8E4 = mybir.dt.float8e4
AF = mybir.ActivationFunctionType

# Q/K projections run in fp8e4m3 DoubleRow (K=256 per matmul).  Weights are
# host-scaled by W8SCALE to sit in e4m3's normal range; the drain divides out.
W8SCALE = 64.0

# IEEE bit-trick exp2 in bf16: exp(SCALE*x) ~= bits_as_bf16(AEXP*x + BEXP).
LOG2E = 1.4426950408889634
AEXP = float(2**7 * LOG2E) * SCALE
BEXP = float(2**7 * (127 - 0.0434609) + 0.5)

RUNAHEAD = 3  # QK chunks in flight ahead of AV (PSUM sc ring = 3)

def dve_exp_chunk(u, c):
    """Which chunks' exp runs on VectorE (bit-exp) vs ScalarE (true exp)."""
    return (c + u) % 2 == 0


def build_nc():
    nc = bacc.Bacc(
        "TRN2",
        target_bir_lowering=False,
        debug=False,
        num_devices=N_CORES,
    )

    # DRAM parameters (per-core shapes; host passes per-core slices).
    # x* are transposed on host: [1024, 512].  Q/K path is fp8 (DoubleRow):
    # w*8 = w.T * W8SCALE in e4m3, b*64 = bias * W8SCALE in bf16.
    # wv/wo are w.T augmented with the bias as row 1024: [1025, 1024].
    xq = nc.dram_tensor("xq", [E, ROWS], F8E4, kind="ExternalInput").ap()
    xk = nc.dram_tensor("xk", [E, ROWS], F8E4, kind="ExternalInput").ap()
    xv = nc.dram_tensor("xv", [E, ROWS], BF16, kind="ExternalInput").ap()
    wq = nc.dram_tensor("wq", [E, E], F8E4, kind="ExternalInput").ap()
    wk = nc.dram_tensor("wk", [E, E], F8E4, kind="ExternalInput").ap()
    bq = nc.dram_tensor("bq", [1, E], BF16, kind="ExternalInput").ap()
    bk = nc.dram_tensor("bk", [1, E], BF16, kind="ExternalInput").ap()
    wv = nc.dram_tensor("wv", [E + 1, E], BF16, kind="ExternalInput").ap()
    wo = nc.dram_tensor("wo", [E + 1, E], BF16, kind="ExternalInput").ap()
    y = nc.dram_tensor("y", [ROWS, E], F32, kind="ExternalOutput").ap()

    with tile.TileContext(nc) as tc:
        build_tile_kernel(tc, xq, xk, xv, wq, wk, bq, bk, wv, wo, y)

    nc.compile()
    return nc


def load_w_chunks(pool, nc, wdram, name):
    """DMA a [1024, 1024] weight as 8 per-k-chunk [128, 1024] tiles on the
    sync (HWDGE) queue; fine granularity lets matmuls start per chunk."""
    tiles = []
    for k in range(8):
        t = pool.tile([128, E], BF16, tag=f"{name}{k}", name=f"{name}{k}")
        nc.sync.dma_start(out=t, in_=wdram[128 * k : 128 * k + 128, :])
        tiles.append(t)
    return tiles


def load_x_chunks(pool, nc, xdram, name):
    tiles = []
    for k in range(8):
        t = pool.tile([128, ROWS], BF16, tag=f"{name}{k}", name=f"{name}{k}")
        nc.gpsimd.dma_start(out=t, in_=xdram[128 * k : 128 * k + 128, :])
        tiles.append(t)
    return tiles


def load_w8_chunks(pool, nc, wdram, name):
    """fp8 weight as 4 DoubleRow superchunks [128, 2, 1024]: partition j of
    chunk k holds contract rows 256k+2j (o=0) and 256k+2j+1 (o=1)."""
    tiles = []
    for k in range(4):
        t = pool.tile([128, 2, E], F8E4, tag=f"{name}{k}", name=f"{name}{k}")
        nc.sync.dma_start(
            out=t,
            in_=wdram[256 * k : 256 * k + 256, :].rearrange("(j o) f -> j o f", o=2),
        )
        tiles.append(t)
    return tiles


def load_x8_chunks(pool, nc, xdram, name):
    tiles = []
    for k in range(4):
        t = pool.tile([128, 2, ROWS], F8E4, tag=f"{name}{k}", name=f"{name}{k}")
        nc.gpsimd.dma_start(
            out=t,
            in_=xdram[256 * k : 256 * k + 256, :].rearrange("(j o) m -> j o m", o=2),
        )
        tiles.append(t)
    return tiles


def build_tile_kernel(tc, xq, xk, xv, wq, wk, bq, bk, wv, wo, y):
    nc = tc.nc

    with (
        tc.tile_pool(name="persist", bufs=1) as persist,
        tc.tile_pool(name="expp", bufs=6) as expp,
        tc.tile_pool(name="tailp", bufs=4) as tailp,
        tc.tile_pool(name="outp", bufs=2) as outp,
        tc.tile_pool(name="dramp", bufs=4, space="DRAM") as dramp,
        # PSUM: tag "sc" 3 x [128,2,512]f32 (6 banks) + tag "av" 2 x 1 bank
        tc.tile_pool(name="ps", bufs=3, space="PSUM") as ps,
    ):
        # ---------------- prefetch everything up-front ----------------
        wq_sb = load_w8_chunks(persist, nc, wq, "wq")
        xq_sb = load_x8_chunks(persist, nc, xq, "xq")
        # bias rows, contiguous + tiny (Q/K pre-scaled by W8SCALE on host)
        bq_row = persist.tile([1, E], BF16, tag="bq_row")
        bk_row = persist.tile([1, E], BF16, tag="bk_row")
        bv_row = persist.tile([1, E], BF16, tag="bv_row")
        bo_row = persist.tile([1, E], BF16, tag="bo_row")
        nc.gpsimd.dma_start(out=bq_row, in_=bq)
        nc.gpsimd.dma_start(out=bk_row, in_=bk)
        nc.gpsimd.dma_start(out=bv_row, in_=wv[E : E + 1, :])
        nc.gpsimd.dma_start(out=bo_row, in_=wo[E : E + 1, :])
        wk_sb = load_w8_chunks(persist, nc, wk, "wk")
        xk_sb = load_x8_chunks(persist, nc, xk, "xk")
        wv_sb = load_w_chunks(persist, nc, wv, "wv")
        xv_sb = load_x_chunks(persist, nc, xv, "xv")
        wo_sb = load_w_chunks(persist, nc, wo, "wo")

        # ---------------- persistent SBUF tensors ----------------
        # qT/kT: [128, pair, r, a]; head h lives at partitions 64*(h%2)..+64,
        # pair index h//2.  Value at [64*(h%2)+d, h//2, r, a] = proj[128h+a, 64r+d].
        qT = persist.tile([128, 2, 16, 128], BF16)
        kT = persist.tile([128, 2, 16, 128], BF16)
        # vones per head: [128(a), 16(r), 65]; [...,:64] = vp rows, [...,64] = 1.0
        vones = [
            persist.tile([128, 16, D + 1], BF16, tag=f"vones{h}", name=f"vones{h}")
            for h in range(4)
        ]
        # oT: attention output, transposed for the out-projection:
        # [128(e%128), 8(e//128), 512(m)]  where e = 64r+d, m = 128h+a.
        oT = persist.tile([128, 8, ROWS], BF16)
        for h in range(4):
            nc.vector.memset(vones[h][:, :, D : D + 1], 1.0)
        x_ones = persist.tile([1, ROWS], BF16, tag="x_ones")
        nc.vector.memset(x_ones, 1.0)

        # broadcast f32 biases for the V / out-proj drains
        bv_bc = persist.tile([128, E], F32, tag="bv_bc")
        bo_bc = persist.tile([128, E], F32, tag="bo_bc")
        bv_f = persist.tile([1, E], F32, tag="bv_f")
        bo_f = persist.tile([1, E], F32, tag="bo_f")
        nc.vector.tensor_copy(bv_f, bv_row)
        nc.vector.tensor_copy(bo_f, bo_row)
        nc.gpsimd.partition_broadcast(bv_bc, bv_f)
        nc.gpsimd.partition_broadcast(bo_bc, bo_f)

        # ---------------- Q / K projections (transposed layout) ----------
        proj_transposed(tc, ps, persist, wq_sb, xq_sb, bq_row, x_ones, qT, "q")
        proj_transposed(tc, ps, persist, wk_sb, xk_sb, bk_row, x_ones, kT, "k")

        # ---------------- V projection (natural layout into vones) -------
        for h in range(4):
            sct = ps.tile([128, 2, ROWS], F32, tag="sc", name=f"accv{h}")
            accs = [sct[:, 0, :], sct[:, 1, :]]
            for k in range(8):
                for g in range(2):
                    nc.tensor.matmul(
                        accs[g],
                        xv_sb[k][:, 128 * h : 128 * h + 128],
                        wv_sb[k][:, 512 * g : 512 * g + 512],
                        start=(k == 0),
                        stop=(k == 7),
                    )
            for g in range(2):
                nc.vector.tensor_tensor(
                    out=vones[h][:, 8 * g : 8 * g + 8, 0:D],
                    in0=accs[g].rearrange("p (rr d) -> p rr d", d=D),
                    in1=bv_bc[:, 512 * g : 512 * g + 512].rearrange(
                        "p (rr d) -> p rr d", d=D
                    ),
                    op=mybir.AluOpType.add,
                )

        # ---------------- attention: 8 units of (head pair, 512 queries) --
        # Tail stages (reciprocal / broadcast / normalize) are deferred into
        # a FIFO and dripped one-per-chunk into the NEXT unit, so each stage's
        # inputs are long since ready when it reaches its engine's strict
        # FIFO head — a stage never parks an engine mid-attention.
        tail_stages = []
        for pr in range(2):
            for iq in range(4):
                attention_unit(
                    tc, ps, expp, tailp, dramp, qT, kT, vones, oT, pr, iq,
                    tail_stages,
                )
        # ---------------- output projection ----------------
        # The last unit's remaining tail stages (heads 2/3) MUST be emitted
        # before the mb matmuls that read their oT slices (Tile deps follow
        # emission order); drip each half's 4 stages before mb1/mb2 so they
        # finish while the earlier mb groups run on the PE.
        for mb in range(4):
            if mb in (1, 2):
                for _ in range(4):
                    if tail_stages:
                        tail_stages.pop(0)()
            acc = ps.tile([128, 2, ROWS], F32, tag="sc", name=f"acco{mb}")
            for v in range(8):
                for g in range(2):
                    nc.tensor.matmul(
                        acc[:, g, :],
                        oT[:, v, 128 * mb : 128 * mb + 128],
                        wo_sb[v][:, 512 * g : 512 * g + 512],
                        start=(v == 0),
                        stop=(v == 7),
                    )
            y_sb = outp.tile([128, E], F32, tag="ysb", name=f"ysb{mb}")
            for g in range(2):
                nc.vector.tensor_tensor(
                    out=y_sb[:, 512 * g : 512 * g + 512],
                    in0=acc[:, g, :],
                    in1=bo_bc[:, 512 * g : 512 * g + 512],
                    op=mybir.AluOpType.add,
                )
            nc.scalar.dma_start(out=y[128 * mb : 128 * mb + 128, :], in_=y_sb)


def proj_transposed(tc, ps, persist, w_sb, x_sb, bias_row, x_ones, dst, nm):
    """Project x @ w.T into the per-head transposed layout `dst`.

    Feature-block v of the PSUM output holds features n = 128v + 64p + d at
    partition 64p + d (p = upper/lower half), i.e. r = 2v + p.  Head h wants
    its data at partition half h%2, so blocks with p == h%2 copy straight
    through (VectorE) and the other half bounce via a staging tile and two
    partition-shifting SBUF->SBUF DMAs on the vector queue (so they never
    block the weight-prefetch sync queue).  Bias rides a 9th K=1 matmul
    against a memset ones row.
    """
    nc = tc.nc
    stg = persist.tile([128, 8, 2, 128], BF16, tag=f"stg_{nm}", name=f"stg_{nm}")
    for half in range(2):  # feature halves: v = 4*half + vl
        acct = [
            ps.tile([128, 2, ROWS], F32, tag="sc", name=f"acc{nm}{half}{t}")
            for t in range(2)
        ]
        accs = [acct[0][:, 0, :], acct[0][:, 1, :], acct[1][:, 0, :], acct[1][:, 1, :]]
        for k in range(5):  # k-outer so compute streams behind the w DMA
            for vl in range(4):
                v = 4 * half + vl
                if k < 4:
                    nc.tensor.matmul(
                        accs[vl],
                        w_sb[k][:, :, 128 * v : 128 * v + 128],
                        x_sb[k],
                        start=(k == 0),
                        stop=False,
                        perf_mode=mybir.MatmulPerfMode.DoubleRow,
                    )
                else:
                    nc.tensor.matmul(
                        accs[vl],
                        bias_row[:, 128 * v : 128 * v + 128],
                        x_ones,
                        start=False,
                        stop=True,
                    )
        # Drains merge both feature-blocks of an acc tile into one 4-D-AP op
        # (512 free elems instead of 256): half the op count on the serial
        # VectorE chain, so the next projection's acc slots free sooner.
        # For v = v0+j (j = the acc tile's vl pair index), r = 2(v0+j)+p is
        # the dst dim-2 slice [2*v0+p : +3 : 2]; the direct copy's middle
        # dims transpose (pair<->h, r<->vl) via the rearranged source AP.
        for t in range(2):
            v0 = 4 * half + 2 * t
            srcT = acct[t].rearrange("d vl (h a) -> d h vl a", a=128)
            srcS = acct[t].rearrange("d vl (h a) -> d vl h a", a=128)
            for p in range(2):
                # heads with h%2 == p whose data sits in psum half q:
                #   q == p   -> direct copy to dst[64p:64p+64, :, 2v+p, :]
                #   q == 1-p -> staging (partition-shift later via DMA)
                # copies divide out the host-side W8SCALE
                nc.vector.tensor_scalar(
                    out=dst[64 * p : 64 * p + 64, :, 2 * v0 + p : 2 * v0 + p + 3 : 2, :],
                    in0=srcT[64 * p : 64 * p + 64, p::2, :, :],
                    scalar1=1.0 / W8SCALE,
                    scalar2=None,
                    op0=mybir.AluOpType.mult,
                )
                q = 1 - p
                nc.vector.tensor_scalar(
                    out=stg[64 * q : 64 * q + 64, 2 * t + 4 * half : 2 * t + 4 * half + 2, :, :],
                    in0=srcS[64 * q : 64 * q + 64, :, p::2, :],
                    scalar1=1.0 / W8SCALE,
                    scalar2=None,
                    op0=mybir.AluOpType.mult,
                )
    for pr in range(2):
        # staged upper half (q=1): r = 2v+1 data for even-parity heads -> lower dst half
        nc.scalar.dma_start(out=dst[0:64, pr, 1::2, :], in_=stg[64:128, :, pr, :])
        # staged lower half (q=0): r = 2v data for odd-parity heads -> upper dst half
        nc.scalar.dma_start(out=dst[64:128, pr, 0::2, :], in_=stg[0:64, :, pr, :])


def attention_unit(tc, ps, expp, tailp, dramp, qT, kT, vones, oT, pr, iq, tail_stages):
    """Heads (2pr, 2pr+1) x queries i' in [512*iq, 512*iq+512).

    Per key-chunk c (128 keys): QK pair (row-strip packed, concurrent),
    one whole-chunk exp on ScalarE or VectorE (alternating), AV pair into
    the per-half [65, 512] accumulators (row 64 = softmax denominator via
    the vones ones-column).  QK runs RUNAHEAD chunks ahead of AV so the PE
    never waits on exp; the sc ring (3 tiles) makes that legal.
    """
    nc = tc.nc
    u = 4 * pr + iq
    qslice = slice(4 * iq, 4 * iq + 4)  # r-blocks of this query window

    av = [
        ps.tile([D + 1, ROWS], F32, tag="av", name=f"av{u}{half}", bufs=2)
        for half in range(2)
    ]
    ex = {}

    def emit_qk_exp(c):
        sc = ps.tile([128, 2, ROWS], F32, tag="sc", name=f"sc{u}{c}")
        for half in range(2):
            base = 64 * half
            nc.tensor.matmul(
                sc[:, half, :],
                kT[base : base + 64, pr, c, :],
                qT[base : base + 64, pr, qslice, :],
                start=True,
                stop=True,
                tile_position=(base, 0),
            )
        sc_flat = sc.rearrange("p h m -> p (h m)")
        if not dve_exp_chunk(u, c):
            e = expp.tile([128, 2, ROWS], BF16, tag="ex", name=f"ex{u}{c}")
            nc.scalar.activation(
                e.rearrange("p h m -> p (h m)"), sc_flat, AF.Exp, scale=SCALE
            )
        else:
            ei = expp.tile([128, 2, ROWS], I16, tag="ex", name=f"exi{u}{c}")
            nc.vector.tensor_scalar(
                out=ei.rearrange("p h m -> p (h m)"),
                in0=sc_flat,
                scalar1=AEXP,
                scalar2=BEXP,
                op0=mybir.AluOpType.mult,
                op1=mybir.AluOpType.add,
            )
            e = ei.bitcast(BF16)
        ex[c] = e

    def emit_av(c):
        for half in range(2):
            h = 2 * pr + half
            nc.tensor.matmul(
                av[half],
                vones[h][:, c, :],
                ex[c][:, half, :],
                start=(c == 0),
                stop=(c == 15),
            )
        del ex[c]

    for c in range(16):
        emit_qk_exp(c)
        if c >= RUNAHEAD:
            emit_av(c - RUNAHEAD)
        # drip the previous unit's tail stages, starting late enough that
        # the DRAM-bounced denominators are already resident (no FIFO parks)
        if c >= 5 and tail_stages:
            tail_stages.pop(0)()
    for c in range(16 - RUNAHEAD, 16):
        emit_av(c)

    # Drain both accumulators immediately (one merged [65,512] ScalarE copy
    # each) so the "av" PSUM slots free for the next unit, and launch the
    # denominator DRAM bounce now (gpsimd DMA queue only).  Everything that
    # would WAIT on a cross-engine result is pushed into tail_stages and
    # dripped during the next unit.
    avds, den_ts = [], []
    for half in range(2):
        avd = tailp.tile([D + 1, ROWS], F32, tag="avd", name=f"avd{u}{half}")
        nc.scalar.copy(avd, av[half])
        avds.append(avd)
    den_ds = []
    for half in range(2):
        den_d = dramp.tile([1, ROWS], F32, tag="dend", name=f"dend{u}{half}")
        nc.gpsimd.dma_start(out=den_d, in_=avds[half][D : D + 1, :])
        den_ds.append(den_d)
    for half in range(2):
        den_t = tailp.tile([4, 128], F32, tag="dent", name=f"dent{u}{half}")
        nc.gpsimd.dma_start(
            out=den_t, in_=den_ds[half].rearrange("o (t a) -> (o t) a", t=4)
        )
        den_ts.append(den_t)

    def stage_recip(half):
        def run():
            nc.vector.reciprocal_approx_fast(out=den_ts[half], in_=den_ts[half])
        return run

    def stage_rec_d(half, rec_d):
        def run():
            nc.gpsimd.dma_start(
                out=rec_d.rearrange("o (t a) -> (o t) a", t=4), in_=den_ts[half]
            )
        return run

    def stage_rec_bc(half, rec_d, rec_bc):
        def run():
            nc.gpsimd.dma_start(out=rec_bc, in_=rec_d.partition_broadcast(64))
        return run

    def stage_norm(half, rec_bc):
        def run():
            h = 2 * pr + half
            av_r = avds[half][0:D].rearrange("d (rl a) -> d rl a", a=128)
            bc_r = rec_bc.rearrange("d (rl a) -> d rl a", a=128)
            # even rl (r = 4iq+rl even): partitions already correct (e%128 = d)
            nc.gpsimd.tensor_tensor(
                out=oT[0:64, 2 * iq : 2 * iq + 2, 128 * h : 128 * h + 128],
                in0=av_r[:, 0::2, :],
                in1=bc_r[:, 0::2, :],
                op=mybir.AluOpType.mult,
            )
            # odd rl: normalize into staging, partition-shift DMA to oT[64:]
            stg_o = tailp.tile([64, 2, 128], BF16, tag="stgo", name=f"stgo{u}{half}")
            nc.gpsimd.tensor_tensor(
                out=stg_o,
                in0=av_r[:, 1::2, :],
                in1=bc_r[:, 1::2, :],
                op=mybir.AluOpType.mult,
            )
            nc.gpsimd.dma_start(
                out=oT[64:128, 2 * iq : 2 * iq + 2, 128 * h : 128 * h + 128],
                in_=stg_o,
            )
        return run

    for half in range(2):
        rec_d = dramp.tile([1, ROWS], F32, tag="recd", name=f"recd{u}{half}")
        rec_bc = tailp.tile([64, ROWS], F32, tag="recbc", name=f"recbc{u}{half}")
        tail_stages.append(stage_recip(half))
        tail_stages.append(stage_rec_d(half, rec_d))
        tail_stages.append(stage_rec_bc(half, rec_d, rec_bc))
        tail_stages.append(stage_norm(half, rec_bc))


_NC_CACHE = {}


def get_nc():
    if "nc" not in _NC_CACHE:
        _NC_CACHE["nc"] = build_nc()
    return _NC_CACHE["nc"]


def shard_inputs(q, k, v, wq, bq, wk, bk, wv, bv, wo, bo):
    """Build the 8 per-core input maps (host-side transposes/augments)."""

    import ml_dtypes

    bf16 = ml_dtypes.bfloat16
    f8 = ml_dtypes.float8_e4m3fn

    def aug_w(w, b):
        return np.concatenate(
            [np.ascontiguousarray(np.asarray(w, np.float32).T),
             np.asarray(b, np.float32)[None, :]],
            axis=0,
        ).astype(bf16)

    def w8(w):
        return np.ascontiguousarray(
            np.asarray(w, np.float32).T * np.float32(W8SCALE)
        ).astype(f8)

    wq_a, wk_a = w8(wq), w8(wk)
    bq_a = (np.asarray(bq, np.float32) * np.float32(W8SCALE))[None, :].astype(bf16)
    bk_a = (np.asarray(bk, np.float32) * np.float32(W8SCALE))[None, :].astype(bf16)
    wv_a, wo_a = aug_w(wv, bv), aug_w(wo, bo)

    in_maps = []
    for c in range(N_CORES):
        b = c // 4
        r0 = 512 * (c % 4)
        sl = slice(r0, r0 + ROWS)

        def t_x(x, dt):
            return np.ascontiguousarray(np.asarray(x[b, sl, :], np.float32).T).astype(
                dt
            )

        in_maps.append(
            {
                "xq": t_x(q, f8),
                "xk": t_x(k, f8),
                "xv": t_x(v, bf16),
                "wq": wq_a,
                "wk": wk_a,
                "bq": bq_a,
                "bk": bk_a,
                "wv": wv_a,
                "wo": wo_a,
            }
        )
    return in_maps


def assemble_output(results):
    out = np.empty((B, S, E), np.float32)
    for c in range(N_CORES):
        b = c // 4
        r0 = 512 * (c % 4)
        out[b, r0 : r0 + ROWS, :] = results[c]["y"]
    return out


def kernel(q, k, v, wq, bq, wk, bk, wv, bv, wo, bo, **run_kwargs):
    nc = get_nc()
    in_maps = shard_inputs(q, k, v, wq, bq, wk, bk, wv, bv, wo, bo)
    res = run_bass_kernel_spmd(nc, in_maps, list(range(N_CORES)), **run_kwargs)
    out = assemble_output(res.results)
    if run_kwargs:
        return out, res
    return out


# revision 59
# speedup vs baseline: 1.0267x; 1.0267x over previous
"""MultiHeadAttention Trainium2 kernel (8-core SPMD, head/batch sharded).

Reference semantics (E=1024, H=16, D=64, B=2, S=2048):
    qp = (q @ wq.T + bq).reshape(B, H, S, D)   # RAW view, not transpose!
    scores = qp @ kp^T * 1/sqrt(E); attn = softmax(scores)
    out = (attn @ vp).reshape(B, S, E) @ wo.T + bo

Because the reshape is a raw view, head h of batch b corresponds to the
contiguous 128-row block rows[128h:128h+128] of the projected [S, E]
matrix, viewed as [2048, 64].  Each core therefore only needs 512 rows of
q/k/v (4 heads) plus the full weight matrices.

Inside each head we use the permuted sequence order i' = 128r + a
(original in-head index i = 16a + r, a=row-in-block 0..127, r=col-block
0..15).  This is a symmetric permutation of Q/K/V rows, so softmax+AV
commute with it; it makes every layout matmul-native.

v2 pipeline notes (vs the v1 baseline at 354us):
  * v1 kept the PE waiting on exp every score chunk (PSUM ring depth 1),
    so the HAM clock gate never saw 3.4us of unbroken PE work and the
    whole attention phase ran at K=4/8 (1.2 GHz).  v2 processes 512
    queries per unit so a score chunk is [128, 2, 512] = 2 PSUM banks,
    the ring holds 3 chunks, and QK runs RUNAHEAD chunks ahead of AV.
  * exp alternates whole chunks between ScalarE (true exp) and VectorE
    (bit-trick exp2); softmax tails use one merged [65, 512] drain copy
    plus reciprocal_approx_fast, with the normalize mults on GpSimd.
  * all weights/x prefetch up-front in per-k-chunk tiles: weights on the
    sync queue, x + small tensors on gpsimd, staging shuffles on the
    vector queue, y stores on scalar — no queue blocks another phase.
  * biases ride the PSUM drain copies (per-partition tensor_scalar for
    the transposed Q/K layout, partition-broadcast tensor_tensor for
    V/out), not 512-cycle K=1 matmuls.
"""

import numpy as np

import concourse.bass as bass
import concourse.mybir as mybir
import concourse.tile as tile
from concourse import bacc
from concourse.bass_utils import run_bass_kernel_spmd

B, S, E = 2, 2048, 1024
H, D = 16, 64
HEADS_PER_CORE = 4
ROWS = 512  # rows of the [S,E] projected matrix handled per core
N_CORES = 8
SCALE = 1.0 / float(np.sqrt(np.float32(E)))

F32 = mybir.dt.float32
BF16 = mybir.dt.bfloat16
I16 = mybir.dt.int16
F8E4 = mybir.dt.float8e4
AF = mybir.ActivationFunctionType

# Q/K projections run in fp8e4m3 DoubleRow (K=256 per matmul).  Weights are
# host-scaled by W8SCALE to sit in e4m3's normal range; the drain divides out.
W8SCALE = 64.0

# IEEE bit-trick exp2 in bf16: exp(SCALE*x) ~= bits_as_bf16(AEXP*x + BEXP).
LOG2E = 1.4426950408889634
AEXP = float(2**7 * LOG2E) * SCALE
BEXP = float(2**7 * (127 - 0.0434609) + 0.5)

RUNAHEAD = 3  # QK chunks in flight ahead of AV (PSUM sc ring = 3)

def dve_exp_chunk(u, c):
    """Which chunks' exp runs on VectorE (bit-exp) vs ScalarE (true exp)."""
    return (c + u) % 2 == 0


def build_nc():
    nc = bacc.Bacc(
        "TRN2",
        target_bir_lowering=False,
        debug=False,
        num_devices=N_CORES,
    )

    # DRAM parameters (per-core shapes; host passes per-core slices).
    # x* are transposed on host: [1024, 512].  Q/K path is fp8 (DoubleRow):
    # w*8 = w.T * W8SCALE in e4m3, b*64 = bias * W8SCALE in bf16.
    # wv/wo are w.T augmented with the bias as row 1024: [1025, 1024].
    xq = nc.dram_tensor("xq", [E, ROWS], F8E4, kind="ExternalInput").ap()
    xk = nc.dram_tensor("xk", [E, ROWS], F8E4, kind="ExternalInput").ap()
    xv = nc.dram_tensor("xv", [E, ROWS], BF16, kind="ExternalInput").ap()
    wq = nc.dram_tensor("wq", [E, E], F8E4, kind="ExternalInput").ap()
    wk = nc.dram_tensor("wk", [E, E], F8E4, kind="ExternalInput").ap()
    bq = nc.dram_tensor("bq", [1, E], BF16, kind="ExternalInput").ap()
    bk = nc.dram_tensor("bk", [1, E], BF16, kind="ExternalInput").ap()
    wv = nc.dram_tensor("wv", [E + 1, E], BF16, kind="ExternalInput").ap()
    wo = nc.dram_tensor("wo", [E + 1, E], BF16, kind="ExternalInput").ap()
    y = nc.dram_tensor("y", [ROWS, E], F32, kind="ExternalOutput").ap()

    with tile.TileContext(nc) as tc:
        build_tile_kernel(tc, xq, xk, xv, wq, wk, bq, bk, wv, wo, y)

    nc.compile()
    return nc


def load_w_chunks(pool, nc, wdram, name):
    """DMA a [1024, 1024] weight as 8 per-k-chunk [128, 1024] tiles on the
    sync (HWDGE) queue; fine granularity lets matmuls start per chunk."""
    tiles = []
    for k in range(8):
        t = pool.tile([128, E], BF16, tag=f"{name}{k}", name=f"{name}{k}")
        nc.sync.dma_start(out=t, in_=wdram[128 * k : 128 * k + 128, :])
        tiles.append(t)
    return tiles


def load_x_chunks(pool, nc, xdram, name):
    tiles = []
    for k in range(8):
        t = pool.tile([128, ROWS], BF16, tag=f"{name}{k}", name=f"{name}{k}")
        nc.gpsimd.dma_start(out=t, in_=xdram[128 * k : 128 * k + 128, :])
        tiles.append(t)
    return tiles


def load_w8_chunks(pool, nc, wdram, name):
    """fp8 weight as 4 DoubleRow superchunks [128, 2, 1024]: partition j of
    chunk k holds contract rows 256k+2j (o=0) and 256k+2j+1 (o=1)."""
    tiles = []
    for k in range(4):
        t = pool.tile([128, 2, E], F8E4, tag=f"{name}{k}", name=f"{name}{k}")
        nc.sync.dma_start(
            out=t,
            in_=wdram[256 * k : 256 * k + 256, :].rearrange("(j o) f -> j o f", o=2),
        )
        tiles.append(t)
    return tiles


def load_x8_chunks(pool, nc, xdram, name):
    tiles = []
    for k in range(4):
        t = pool.tile([128, 2, ROWS], F8E4, tag=f"{name}{k}", name=f"{name}{k}")
        nc.gpsimd.dma_start(
            out=t,
            in_=xdram[256 * k : 256 * k + 256, :].rearrange("(j o) m -> j o m", o=2),
        )
        tiles.append(t)
    return tiles


def build_tile_kernel(tc, xq, xk, xv, wq, wk, bq, bk, wv, wo, y):
    nc = tc.nc

    with (
        tc.tile_pool(name="persist", bufs=1) as persist,
        tc.tile_pool(name="expp", bufs=6) as expp,
        tc.tile_pool(name="tailp", bufs=4) as tailp,
        tc.tile_pool(name="outp", bufs=2) as outp,
        tc.tile_pool(name="dramp", bufs=4, space="DRAM") as dramp,
        # PSUM: tag "sc" 3 x [128,2,512]f32 (6 banks) + tag "av" 2 x 1 bank
        tc.tile_pool(name="ps", bufs=3, space="PSUM") as ps,
    ):
        # ---------------- prefetch everything up-front ----------------
        wq_sb = load_w8_chunks(persist, nc, wq, "wq")
        xq_sb = load_x8_chunks(persist, nc, xq, "xq")
        # bias rows, contiguous + tiny (Q/K pre-scaled by W8SCALE on host)
        bq_row = persist.tile([1, E], BF16, tag="bq_row")
        bk_row = persist.tile([1, E], BF16, tag="bk_row")
        bv_row = persist.tile([1, E], BF16, tag="bv_row")
        bo_row = persist.tile([1, E], BF16, tag="bo_row")
        nc.gpsimd.dma_start(out=bq_row, in_=bq)
        nc.gpsimd.dma_start(out=bk_row, in_=bk)
        nc.gpsimd.dma_start(out=bv_row, in_=wv[E : E + 1, :])
        nc.gpsimd.dma_start(out=bo_row, in_=wo[E : E + 1, :])
        wk_sb = load_w8_chunks(persist, nc, wk, "wk")
        xk_sb = load_x8_chunks(persist, nc, xk, "xk")
        wv_sb = load_w_chunks(persist, nc, wv, "wv")
        xv_sb = load_x_chunks(persist, nc, xv, "xv")
        wo_sb = load_w_chunks(persist, nc, wo, "wo")

        # ---------------- persistent SBUF tensors ----------------
        # qT/kT: [128, pair, r, a]; head h lives at partitions 64*(h%2)..+64,
        # pair index h//2.  Value at [64*(h%2)+d, h//2, r, a] = proj[128h+a, 64r+d].
        qT = persist.tile([128, 2, 16, 128], BF16)
        kT = persist.tile([128, 2, 16, 128], BF16)
        # vones per head: [128(a), 16(r), 65]; [...,:64] = vp rows, [...,64] = 1.0
        vones = [
            persist.tile([128, 16, D + 1], BF16, tag=f"vones{h}", name=f"vones{h}")
            for h in range(4)
        ]
        # oT: attention output, transposed for the out-projection:
        # [128(e%128), 8(e//128), 512(m)]  where e = 64r+d, m = 128h+a.
        oT = persist.tile([128, 8, ROWS], BF16)
        for h in range(4):
            nc.vector.memset(vones[h][:, :, D : D + 1], 1.0)
        x_ones = persist.tile([1, ROWS], BF16, tag="x_ones")
        nc.vector.memset(x_ones, 1.0)

        # broadcast f32 biases for the V / out-proj drains
        bv_bc = persist.tile([128, E], F32, tag="bv_bc")
        bo_bc = persist.tile([128, E], F32, tag="bo_bc")
        bv_f = persist.tile([1, E], F32, tag="bv_f")
        bo_f = persist.tile([1, E], F32, tag="bo_f")
        nc.vector.tensor_copy(bv_f, bv_row)
        nc.vector.tensor_copy(bo_f, bo_row)
        nc.gpsimd.partition_broadcast(bv_bc, bv_f)
        nc.gpsimd.partition_broadcast(bo_bc, bo_f)

        # ---------------- Q / K projections (transposed layout) ----------
        proj_transposed(tc, ps, persist, wq_sb, xq_sb, bq_row, x_ones, qT, "q")
        proj_transposed(tc, ps, persist, wk_sb, xk_sb, bk_row, x_ones, kT, "k")

        # ---------------- V projection (natural layout into vones) -------
        for h in range(4):
            sct = ps.tile([128, 2, ROWS], F32, tag="sc", name=f"accv{h}")
            accs = [sct[:, 0, :], sct[:, 1, :]]
            for k in range(8):
                for g in range(2):
                    nc.tensor.matmul(
                        accs[g],
                        xv_sb[k][:, 128 * h : 128 * h + 128],
                        wv_sb[k][:, 512 * g : 512 * g + 512],
                        start=(k == 0),
                        stop=(k == 7),
                    )
            for g in range(2):
                nc.vector.tensor_tensor(
                    out=vones[h][:, 8 * g : 8 * g + 8, 0:D],
                    in0=accs[g].rearrange("p (rr d) -> p rr d", d=D),
                    in1=bv_bc[:, 512 * g : 512 * g + 512].rearrange(
                        "p (rr d) -> p rr d", d=D
                    ),
                    op=mybir.AluOpType.add,
                )

        # ---------------- attention: 8 units of (head pair, 512 queries) --
        # Tail stages (reciprocal / broadcast / normalize) are deferred into
        # a FIFO and dripped one-per-chunk into the NEXT unit, so each stage's
        # inputs are long since ready when it reaches its engine's strict
        # FIFO head — a stage never parks an engine mid-attention.
        tail_stages = []
        for pr in range(2):
            for iq in range(4):
                attention_unit(
                    tc, ps, expp, tailp, dramp, qT, kT, vones, oT, pr, iq,
                    tail_stages,
                )
        # ---------------- output projection ----------------
        # The last unit's remaining tail stages (heads 2/3) MUST be emitted
        # before the mb matmuls that read their oT slices (Tile deps follow
        # emission order); drip each half's 4 stages before mb1/mb2 so they
        # finish while the earlier mb groups run on the PE.
        for mb in range(4):
            if mb in (1, 2):
                for _ in range(4):
                    if tail_stages:
                        tail_stages.pop(0)()
            acc = ps.tile([128, 2, ROWS], F32, tag="sc", name=f"acco{mb}")
            for v in range(8):
                for g in range(2):
                    nc.tensor.matmul(
                        acc[:, g, :],
                        oT[:, v, 128 * mb : 128 * mb + 128],
                        wo_sb[v][:, 512 * g : 512 * g + 512],
                        start=(v == 0),
                        stop=(v == 7),
                    )
            y_sb = outp.tile([128, E], F32, tag="ysb", name=f"ysb{mb}")
            for g in range(2):
                nc.vector.tensor_tensor(
                    out=y_sb[:, 512 * g : 512 * g + 512],
                    in0=acc[:, g, :],
                    in1=bo_bc[:, 512 * g : 512 * g + 512],
                    op=mybir.AluOpType.add,
                )
            nc.scalar.dma_start(out=y[128 * mb : 128 * mb + 128, :], in_=y_sb)


def proj_transposed(tc, ps, persist, w_sb, x_sb, bias_row, x_ones, dst, nm):
    """Project x @ w.T into the per-head transposed layout `dst`.

    Feature-block v of the PSUM output holds features n = 128v + 64p + d at
    partition 64p + d (p = upper/lower half), i.e. r = 2v + p.  Head h wants
    its data at partition half h%2, so blocks with p == h%2 copy straight
    through (VectorE) and the other half bounce via a staging tile and two
    partition-shifting SBUF->SBUF DMAs on the vector queue (so they never
    block the weight-prefetch sync queue).  Bias rides a 9th K=1 matmul
    against a memset ones row.
    """
    nc = tc.nc
    stg = persist.tile([128, 8, 2, 128], BF16, tag=f"stg_{nm}", name=f"stg_{nm}")
    for half in range(2):  # feature halves: v = 4*half + vl
        acct = [
            ps.tile([128, 2, ROWS], F32, tag="sc", name=f"acc{nm}{half}{t}")
            for t in range(2)
        ]
        accs = [acct[0][:, 0, :], acct[0][:, 1, :], acct[1][:, 0, :], acct[1][:, 1, :]]
        for k in range(5):  # k-outer so compute streams behind the w DMA
            for vl in range(4):
                v = 4 * half + vl
                if k < 4:
                    nc.tensor.matmul(
                        accs[vl],
                        w_sb[k][:, :, 128 * v : 128 * v + 128],
                        x_sb[k],
                        start=(k == 0),
                        stop=False,
                        perf_mode=mybir.MatmulPerfMode.DoubleRow,
                    )
                else:
                    nc.tensor.matmul(
                        accs[vl],
                        bias_row[:, 128 * v : 128 * v + 128],
                        x_ones,
                        start=False,
                        stop=True,
                    )
        for vl in range(4):
            v = 4 * half + vl
            src = accs[vl].rearrange("p (h a) -> p h a", a=128)
            for p in range(2):
                # heads with h%2 == p whose data sits in psum half q:
                #   q == p   -> direct copy to dst[64p:64p+64, :, 2v+p, :]
                #   q == 1-p -> staging (partition-shift later via DMA)
                # copies divide out the host-side W8SCALE
                nc.vector.tensor_scalar(
                    out=dst[64 * p : 64 * p + 64, :, 2 * v + p, :],
                    in0=src[64 * p : 64 * p + 64, p::2, :],
                    scalar1=1.0 / W8SCALE,
                    scalar2=None,
                    op0=mybir.AluOpType.mult,
                )
                q = 1 - p
                nc.vector.tensor_scalar(
                    out=stg[64 * q : 64 * q + 64, v, :, :],
                    in0=src[64 * q : 64 * q + 64, p::2, :],
                    scalar1=1.0 / W8SCALE,
                    scalar2=None,
                    op0=mybir.AluOpType.mult,
                )
    for pr in range(2):
        # staged upper half (q=1): r = 2v+1 data for even-parity heads -> lower dst half
        nc.scalar.dma_start(out=dst[0:64, pr, 1::2, :], in_=stg[64:128, :, pr, :])
        # staged lower half (q=0): r = 2v data for odd-parity heads -> upper dst half
        nc.scalar.dma_start(out=dst[64:128, pr, 0::2, :], in_=stg[0:64, :, pr, :])


def attention_unit(tc, ps, expp, tailp, dramp, qT, kT, vones, oT, pr, iq, tail_stages):
    """Heads (2pr, 2pr+1) x queries i' in [512*iq, 512*iq+512).

    Per key-chunk c (128 keys): QK pair (row-strip packed, concurrent),
    one whole-chunk exp on ScalarE or VectorE (alternating), AV pair into
    the per-half [65, 512] accumulators (row 64 = softmax denominator via
    the vones ones-column).  QK runs RUNAHEAD chunks ahead of AV so the PE
    never waits on exp; the sc ring (3 tiles) makes that legal.
    """
    nc = tc.nc
    u = 4 * pr + iq
    qslice = slice(4 * iq, 4 * iq + 4)  # r-blocks of this query window

    av = [
        ps.tile([D + 1, ROWS], F32, tag="av", name=f"av{u}{half}", bufs=2)
        for half in range(2)
    ]
    ex = {}

    def emit_qk_exp(c):
        sc = ps.tile([128, 2, ROWS], F32, tag="sc", name=f"sc{u}{c}")
        for half in range(2):
            base = 64 * half
            nc.tensor.matmul(
                sc[:, half, :],
                kT[base : base + 64, pr, c, :],
                qT[base : base + 64, pr, qslice, :],
                start=True,
                stop=True,
                tile_position=(base, 0),
            )
        sc_flat = sc.rearrange("p h m -> p (h m)")
        if not dve_exp_chunk(u, c):
            e = expp.tile([128, 2, ROWS], BF16, tag="ex", name=f"ex{u}{c}")
            nc.scalar.activation(
                e.rearrange("p h m -> p (h m)"), sc_flat, AF.Exp, scale=SCALE
            )
        else:
            ei = expp.tile([128, 2, ROWS], I16, tag="ex", name=f"exi{u}{c}")
            nc.vector.tensor_scalar(
                out=ei.rearrange("p h m -> p (h m)"),
                in0=sc_flat,
                scalar1=AEXP,
                scalar2=BEXP,
                op0=mybir.AluOpType.mult,
                op1=mybir.AluOpType.add,
            )
            e = ei.bitcast(BF16)
        ex[c] = e

    def emit_av(c):
        for half in range(2):
            h = 2 * pr + half
            nc.tensor.matmul(
                av[half],
                vones[h][:, c, :],
                ex[c][:, half, :],
                start=(c == 0),
                stop=(c == 15),
            )
        del ex[c]

    for c in range(16):
        emit_qk_exp(c)
        if c >= RUNAHEAD:
            emit_av(c - RUNAHEAD)
        # drip the previous unit's tail stages, starting late enough that
        # the DRAM-bounced denominators are already resident (no FIFO parks)
        if c >= 5 and tail_stages:
            tail_stages.pop(0)()
    for c in range(16 - RUNAHEAD, 16):
        emit_av(c)

    # Drain both accumulators immediately (one merged [65,512] ScalarE copy
    # each) so the "av" PSUM slots free for the next unit, and launch the
    # denominator DRAM bounce now (gpsimd DMA queue only).  Everything that
    # would WAIT on a cross-engine result is pushed into tail_stages and
    # dripped during the next unit.
    avds, den_ts = [], []
    for half in range(2):
        avd = tailp.tile([D + 1, ROWS], F32, tag="avd", name=f"avd{u}{half}")
        nc.scalar.copy(avd, av[half])
        avds.append(avd)
    den_ds = []
    for half in range(2):
        den_d = dramp.tile([1, ROWS], F32, tag="dend", name=f"dend{u}{half}")
        nc.gpsimd.dma_start(out=den_d, in_=avds[half][D : D + 1, :])
        den_ds.append(den_d)
    for half in range(2):
        den_t = tailp.tile([4, 128], F32, tag="dent", name=f"dent{u}{half}")
        nc.gpsimd.dma_start(
            out=den_t, in_=den_ds[half].rearrange("o (t a) -> (o t) a", t=4)
        )
        den_ts.append(den_t)

    def stage_recip(half):
        def run():
            nc.vector.reciprocal_approx_fast(out=den_ts[half], in_=den_ts[half])
        return run

    def stage_rec_d(half, rec_d):
        def run():
            nc.gpsimd.dma_start(
                out=rec_d.rearrange("o (t a) -> (o t) a", t=4), in_=den_ts[half]
            )
        return run

    def stage_rec_bc(half, rec_d, rec_bc):
        def run():
            nc.gpsimd.dma_start(out=rec_bc, in_=rec_d.partition_broadcast(64))
        return run

    def stage_norm(half, rec_bc):
        def run():
            h = 2 * pr + half
            av_r = avds[half][0:D].rearrange("d (rl a) -> d rl a", a=128)
            bc_r = rec_bc.rearrange("d (rl a) -> d rl a", a=128)
            # even rl (r = 4iq+rl even): partitions already correct (e%128 = d)
            nc.gpsimd.tensor_tensor(
                out=oT[0:64, 2 * iq : 2 * iq + 2, 128 * h : 128 * h + 128],
                in0=av_r[:, 0::2, :],
                in1=bc_r[:, 0::2, :],
                op=mybir.AluOpType.mult,
            )
            # odd rl: normalize into staging, partition-shift DMA to oT[64:]
            stg_o = tailp.tile([64, 2, 128], BF16, tag="stgo", name=f"stgo{u}{half}")
            nc.gpsimd.tensor_tensor(
                out=stg_o,
                in0=av_r[:, 1::2, :],
                in1=bc_r[:, 1::2, :],
                op=mybir.AluOpType.mult,
            )
            nc.gpsimd.dma_start(
                out=oT[64:128, 2 * iq : 2 * iq + 2, 128 * h : 128 * h + 128],
                in_=stg_o,
            )
        return run

    for half in range(2):
        rec_d = dramp.tile([1, ROWS], F32, tag="recd", name=f"recd{u}{half}")
        rec_bc = tailp.tile([64, ROWS], F32, tag="recbc", name=f"recbc{u}{half}")
        tail_stages.append(stage_recip(half))
        tail_stages.append(stage_rec_d(half, rec_d))
        tail_stages.append(stage_rec_bc(half, rec_d, rec_bc))
        tail_stages.append(stage_norm(half, rec_bc))


_NC_CACHE = {}


def get_nc():
    if "nc" not in _NC_CACHE:
        _NC_CACHE["nc"] = build_nc()
    return _NC_CACHE["nc"]


def shard_inputs(q, k, v, wq, bq, wk, bk, wv, bv, wo, bo):
    """Build the 8 per-core input maps (host-side transposes/augments)."""

    import ml_dtypes

    bf16 = ml_dtypes.bfloat16
    f8 = ml_dtypes.float8_e4m3fn

    def aug_w(w, b):
        return np.concatenate(
            [np.ascontiguousarray(np.asarray(w, np.float32).T),
             np.asarray(b, np.float32)[None, :]],
            axis=0,
        ).astype(bf16)

    def w8(w):
        return np.ascontiguousarray(
            np.asarray(w, np.float32).T * np.float32(W8SCALE)
        ).astype(f8)

    wq_a, wk_a = w8(wq), w8(wk)
    bq_a = (np.asarray(bq, np.float32) * np.float32(W8SCALE))[None, :].astype(bf16)
    bk_a = (np.asarray(bk, np.float32) * np.float32(W8SCALE))[None, :].astype(bf16)
    wv_a, wo_a = aug_w(wv, bv), aug_w(wo, bo)

    in_maps = []
    for c in range(N_CORES):
        b = c // 4
        r0 = 512 * (c % 4)
        sl = slice(r0, r0 + ROWS)

        def t_x(x, dt):
            return np.ascontiguousarray(np.asarray(x[b, sl, :], np.float32).T).astype(
                dt
            )

        in_maps.append(
            {
                "xq": t_x(q, f8),
                "xk": t_x(k, f8),
                "xv": t_x(v, bf16),
                "wq": wq_a,
                "wk": wk_a,
                "bq": bq_a,
                "bk": bk_a,
                "wv": wv_a,
                "wo": wo_a,
            }
        )
    return in_maps


def assemble_output(results):
    out = np.empty((B, S, E), np.float32)
    for c in range(N_CORES):
        b = c // 4
        r0 = 512 * (c % 4)
        out[b, r0 : r0 + ROWS, :] = results[c]["y"]
    return out


def kernel(q, k, v, wq, bq, wk, bk, wv, bv, wo, bo, **run_kwargs):
    nc = get_nc()
    in_maps = shard_inputs(q, k, v, wq, bq, wk, bk, wv, bv, wo, bo)
    res = run_bass_kernel_spmd(nc, in_maps, list(range(N_CORES)), **run_kwargs)
    out = assemble_output(res.results)
    if run_kwargs:
        return out, res
    return out
